# revision 1
# baseline (speedup 1.0000x reference)
"""Conv2D 3x3 (NCHW, OIHW, stride 1, pad 1) on 8 Trainium2 NeuronCores.

Problem shape: input (32, 128, 56, 56) fp32, weights (256, 128, 3, 3) fp32,
output (32, 256, 56, 56) fp32.

Strategy:
  - Data-parallel over batch: 4 images per core, weights replicated.
  - Host zero-pads images to 58x58 and re-lays weights as [ci, tap, co]
    so the device kernel is pure shifted matmuls.
  - Per image: for each 8-row output chunk (8x56 = 448 pixels) and each
    co-half (128 of 256), accumulate 9 tap matmuls in PSUM:
        psum[co, pix] += W[tap][ci, co].T @ x_pad[ci, shifted pixels]
    contract dim = 128 channels (full partitions), moving free dim = 448.
  - Operands are bitcast to float32r for the single-pass PE fp32 path.
"""

import sys

sys.path.insert(0, "/opt/trn_rl_repo")

import numpy as np

N_CORES = 8
N_FULL = 32
IMGS = N_FULL // N_CORES  # images per core
CIN = 128
COUT = 256
H = W = 56
HP = WP = 58  # padded
PIX = H * W  # 3136
PPIX = HP * WP  # 3364
ROWS_PER_CHUNK = 8
N_CHUNKS = H // ROWS_PER_CHUNK  # 7
CHUNK = ROWS_PER_CHUNK * W  # 448 moving elements per matmul

_CACHE = {}


def _split_sync_waits(nc, mybir, max_waits=1):
    """The walrus build in this container rejects instructions carrying
    more than one semaphore wait; hoist extras onto preceding NOPs on the
    same engine (engine executes them in order, semantics preserved)."""
    ctr = 0
    for f in nc.m.functions:
        for bb in f.blocks:
            new_insts = []
            for ins in bb.instructions:
                si = getattr(ins, "sync_info", None)
                if si is not None and si.on_wait and len(si.on_wait) > max_waits:
                    waits = list(si.on_wait)
                    extra, keep = waits[:-max_waits], waits[-max_waits:]
                    for i in range(0, len(extra), max_waits):
                        ctr += 1
                        nop = mybir.InstNoOp(
                            name=f"{ins.name}_wsplit{ctr}",
                            engine=ins.engine,
                            sync_info=mybir.SyncInfo(
                                on_wait=extra[i : i + max_waits], on_update=[]
                            ),
                            bass_nofuse=True,
                        )
                        new_insts.append(nop)
                    si.on_wait = keep
                new_insts.append(ins)
            bb.instructions[:] = new_insts
    return ctr


# input row split: chunks 0-3 read padded rows 0..33, chunks 4-6 rows 32..57
ROWS_A = 34  # padded rows 0..33
ROWS_B = HP - 32  # padded rows 32..57 (26 rows)
# output pix split per co-half: chunks 0-3 (1792 pix), chunks 4-6 (1344 pix)
PIX_A = 4 * CHUNK
PIX_B = PIX - PIX_A


def _build():
    import concourse.bass as bass
    import concourse.mybir as mybir
    import concourse.tile as tile

    f32 = mybir.dt.float32
    f16 = mybir.dt.float16

    nc = bass.Bass()
    # x/w converted to fp16 on the host: halves the load DMA bytes and
    # the weight-load (LDWEIGHTS) time that paces the matmul stream.
    x = nc.declare_dram_parameter("x", [IMGS, CIN, PPIX], f16, isOutput=False)
    w = nc.declare_dram_parameter("w", [CIN, 9 * COUT], f16, isOutput=False)
    out = nc.declare_dram_parameter("out", [IMGS, COUT, PIX], f32, isOutput=True)

    x4 = x.rearrange("n p (r c) -> n p r c", c=WP)
    w3 = w.rearrange("p (h k) -> p h k", h=2)  # h-major weight halves

    with tile.TileContext(nc) as tc:
        with (
            tc.tile_pool(name="wpool", bufs=1) as wpool,
            tc.tile_pool(name="xapool", bufs=2) as xapool,
            tc.tile_pool(name="xbpool", bufs=2) as xbpool,
            tc.tile_pool(name="opool", bufs=2) as opool,
            tc.tile_pool(name="psum", bufs=8, space="PSUM") as pspool,
        ):
            # PE warmup: dummy matmuls on a zeroed scratch tile while the
            # first DMAs are in flight, so HAM un-throttles (1.2->2.4 GHz)
            # before the real matmuls start.
            # HAM flips to full clock only after ~3.4us of sustained PE
            # busy; 16 x N=256 cold matmuls (~213ns each) cover that window
            # while the first input DMAs are still in flight.
            warm = wpool.tile([128, 256], f16, name="warm")
            nc.vector.memzero(warm[:])
            wps = pspool.tile([128, 256], f32, name="ps")
            for _ in range(16):
                nc.tensor.matmul(
                    wps[:], lhsT=warm[:, 0:128], rhs=warm[:], start=True, stop=True
                )

            # weights on the scalar HWDGE ring (h0 first so chunk-0 h0
            # matmuls start earliest); images on the sync ring.
            wt = wpool.tile([CIN, 9 * COUT], f16)
            wt3 = wt.rearrange("p (h k) -> p h k", h=2)
            nc.scalar.dma_start(out=wt3[:, 0, :], in_=w3[:, 0, :])
            nc.scalar.dma_start(out=wt3[:, 1, :], in_=w3[:, 1, :])

            for n in range(IMGS):
                xa = xapool.tile([CIN, ROWS_A * WP], f16)
                xb = xbpool.tile([CIN, ROWS_B * WP], f16)
                xa3 = xa.rearrange("p (r c) -> p r c", c=WP)
                xb3 = xb.rearrange("p (r c) -> p r c", c=WP)
                # first 10 rows land first so chunk 0 can start right away
                nc.sync.dma_start(out=xa3[:, 0:10, :], in_=x4[n, :, 0:10, :])
                nc.sync.dma_start(out=xa3[:, 10:ROWS_A, :], in_=x4[n, :, 10:ROWS_A, :])
                nc.sync.dma_start(out=xb[:], in_=x4[n, :, 32:HP, :])

                def rhs(c, dy, dx):
                    if c < 4:
                        return xa3[
                            :,
                            c * ROWS_PER_CHUNK + dy : c * ROWS_PER_CHUNK + dy + ROWS_PER_CHUNK,
                            dx : dx + W,
                        ]
                    r0 = c * ROWS_PER_CHUNK + dy - 32
                    return xb3[:, r0 : r0 + ROWS_PER_CHUNK, dx : dx + W]

                ot = opool.tile([128, 2 * PIX], f32)
                # image 0: lead with a chunk-0-only group so the first real
                # matmuls wait only on the first 10 input rows + w half 0.
                # last image: trail with a chunk-6-only group so the final
                # exposed PSUM-copy + DMA is one small piece.
                if n == 0:
                    grps = ((0,), (1, 2, 3), (4, 5, 6))
                elif n == IMGS - 1:
                    grps = ((0, 1, 2, 3), (4, 5), (6,))
                else:
                    grps = ((0, 1, 2, 3), (4, 5, 6))
                for h in range(2):
                    for grp in grps:
                        pss = {
                            c: pspool.tile([128, CHUNK], f32, name="ps") for c in grp
                        }
                        for tap in range(9):
                            dy, dx = divmod(tap, 3)
                            col0 = h * 9 * 128 + tap * 128
                            for c in grp:
                                nc.tensor.matmul(
                                    pss[c][:],
                                    lhsT=wt[:, col0 : col0 + 128],
                                    rhs=rhs(c, dy, dx),
                                    start=(tap == 0),
                                    stop=(tap == 8),
                                )
                        # copy each finished chunk out of PSUM and stream it
                        # to DRAM immediately (alternating DMA rings so the
                        # final exposed piece is one small transfer)
                        for c in grp:
                            nc.vector.tensor_copy(
                                out=ot[
                                    :, h * PIX + c * CHUNK : h * PIX + (c + 1) * CHUNK
                                ],
                                in_=pss[c][:],
                            )
                            ring = nc.scalar if c % 2 == 0 else nc.sync
                            ring.dma_start(
                                out=out[
                                    n, h * 128 : (h + 1) * 128, c * CHUNK : (c + 1) * CHUNK
                                ],
                                in_=ot[
                                    :, h * PIX + c * CHUNK : h * PIX + (c + 1) * CHUNK
                                ],
                            )

    _split_sync_waits(nc, mybir)
    return nc


def _prep_inputs(input_batch, weights):
    xp = np.zeros((N_FULL, CIN, HP, WP), dtype=np.float16)
    xp[:, :, 1:-1, 1:-1] = input_batch
    xp = xp.reshape(N_FULL, CIN, PPIX)
    # w[ci, h*1152 + tap*128 + c] = weights[h*128 + c, ci, dy, dx]
    wt = np.ascontiguousarray(
        weights.astype(np.float32)
        .transpose(1, 2, 3, 0)  # [ci, dy, dx, co]
        .reshape(CIN, 3, 3, 2, 128)  # co -> (h, c)
        .transpose(0, 3, 1, 2, 4)  # [ci, h, dy, dx, c]
        .reshape(CIN, 9 * COUT)
        .astype(np.float16)
    )
    in_maps = []
    for i in range(N_CORES):
        in_maps.append(
            {
                "x": np.ascontiguousarray(xp[i * IMGS : (i + 1) * IMGS]),
                "w": wt,
            }
        )
    return in_maps


def _run(input_batch, weights, trace=False):
    from concourse.bass_utils import run_bass_kernel_spmd

    if "nc" not in _CACHE:
        _CACHE["nc"] = _build()
    nc = _CACHE["nc"]
    in_maps = _prep_inputs(np.asarray(input_batch), np.asarray(weights))
    res = run_bass_kernel_spmd(nc, in_maps, list(range(N_CORES)), trace=trace)
    outs = [res.results[i]["out"].reshape(IMGS, COUT, H, W) for i in range(N_CORES)]
    full = np.concatenate(outs, axis=0).astype(np.float32)
    return full, res


def kernel(input_batch, weights):
    full, _ = _run(input_batch, weights, trace=False)
    return full



# revision 2
# speedup vs baseline: 1.0268x; 1.0268x over previous
"""Conv2D 3x3 (NCHW, OIHW, stride 1, pad 1) on 8 Trainium2 NeuronCores.

Problem shape: input (32, 128, 56, 56) fp32, weights (256, 128, 3, 3) fp32,
output (32, 256, 56, 56) fp32.

Strategy (v2 — width-axis Winograd F(2,3)):
  - Data-parallel over batch: 4 images per core, weights replicated.
  - Host applies the 1D Winograd F(2,3) input transform along W to the
    zero-padded image: for each padded row r (58 rows) and tile t (28
    2-wide output tiles), V0=d[2t]-d[2t+2], V1=d[2t+1]+d[2t+2],
    V2=d[2t+2]-d[2t+1], V3=d[2t+1]-d[2t+3], stored as 4 fp16 planes of
    [ci, 58*28].  Weights become U[dy,p] = G @ w-taps (G the F(2,3)
    weight transform), fp16.
  - Device: per image, co-half h, and 14-row chunk c, accumulate
        m_p[co, 392] = sum_dy U[h,dy,p][ci,co].T @ V_p[ci, rows 14c+dy]
    (12 matmuls, free dim 392, contract 128) into one PSUM bank per p.
    This is 1.5x fewer PE columns than direct 9-tap conv.
  - Output transform fused into the PSUM drain: ScalarE copies m1,m2 to
    SBUF fp16; VectorE computes Y0 = (m1+m2)+m0 and Y1 = (m1-m2)-m3,
    writing fp16 output planes that DMA out.  Host interleaves the two
    w-phase planes and upcasts to fp32.
"""

import sys

sys.path.insert(0, "/opt/trn_rl_repo")

import numpy as np

N_CORES = 8
N_FULL = 32
IMGS = N_FULL // N_CORES  # images per core
CIN = 128
COUT = 256
H = W = 56
HP = 58  # padded rows
T = 28  # winograd tiles per row (2 output cols each)
NP = 4  # winograd positions per tile
VROW = HP * T  # 1624 elements per V plane
ROWS_PER_CHUNK = 14
N_CHUNKS = H // ROWS_PER_CHUNK  # 4
FD = ROWS_PER_CHUNK * T  # 392 moving elements per matmul
PIX = H * W  # 3136

_CACHE = {}


def _split_sync_waits(nc, mybir, max_waits=1):
    """The walrus build in this container rejects instructions carrying
    more than one semaphore wait; hoist extras onto preceding NOPs on the
    same engine (engine executes them in order, semantics preserved)."""
    ctr = 0
    for f in nc.m.functions:
        for bb in f.blocks:
            new_insts = []
            for ins in bb.instructions:
                si = getattr(ins, "sync_info", None)
                if si is not None and si.on_wait and len(si.on_wait) > max_waits:
                    waits = list(si.on_wait)
                    extra, keep = waits[:-max_waits], waits[-max_waits:]
                    for i in range(0, len(extra), max_waits):
                        ctr += 1
                        nop = mybir.InstNoOp(
                            name=f"{ins.name}_wsplit{ctr}",
                            engine=ins.engine,
                            sync_info=mybir.SyncInfo(
                                on_wait=extra[i : i + max_waits], on_update=[]
                            ),
                            bass_nofuse=True,
                        )
                        new_insts.append(nop)
                    si.on_wait = keep
                new_insts.append(ins)
            bb.instructions[:] = new_insts
    return ctr


# input V-plane row ranges DMA'd per piece (lead piece first so chunk 0
# can start as early as possible)
DMA_ROWS = ((0, 16), (16, 30), (30, 44), (44, 58))


def _build():
    import concourse.bass as bass
    import concourse.mybir as mybir
    import concourse.tile as tile

    f32 = mybir.dt.float32
    f16 = mybir.dt.float16

    nc = bass.Bass()
    x = nc.declare_dram_parameter("x", [IMGS, CIN, NP * VROW], f16, isOutput=False)
    w = nc.declare_dram_parameter("w", [CIN, 2 * 3 * NP * 128], f16, isOutput=False)
    out = nc.declare_dram_parameter("out", [IMGS, COUT, 2 * PIX // 2], f16, isOutput=True)

    x4 = x.rearrange("n p (v q) -> n p v q", v=NP)  # q = 1624 (row*28)
    w5 = w.rearrange("p (h y v c) -> p h y v c", h=2, y=3, v=NP)
    out4 = out.rearrange("n c (v q) -> n c v q", v=2)  # q = 1568 (row*28)

    with tile.TileContext(nc) as tc:
        with (
            tc.tile_pool(name="wpool", bufs=1) as wpool,
            tc.tile_pool(name="xpool", bufs=2) as xpool,
            tc.tile_pool(name="cpool", bufs=4) as cpool,
            tc.tile_pool(name="spool", bufs=4) as spool,
            tc.tile_pool(name="opool", bufs=4) as opool,
            tc.tile_pool(name="psum", bufs=2, space="PSUM") as pspool,
        ):
            # PE warmup: dummy matmuls while the first DMAs are in flight so
            # HAM un-throttles (1.2->2.4 GHz) before the real matmuls start.
            warm = wpool.tile([128, 256], f16, name="warm")
            nc.vector.memzero(warm[:])
            wps = pspool.tile([128, NP, 512], f32, name="ps")
            for _ in range(16):
                nc.tensor.matmul(
                    wps[:, 0, 0:256], lhsT=warm[:, 0:128], rhs=warm[:],
                    start=True, stop=True,
                )

            # weights on the scalar HWDGE ring (h0 first so chunk-0 h0
            # matmuls start earliest); images on the sync ring.
            wt = wpool.tile([CIN, 2 * 3 * NP * 128], f16)
            wt5 = wt.rearrange("p (h y v c) -> p h y v c", h=2, y=3, v=NP)
            nc.scalar.dma_start(out=wt5[:, 0], in_=w5[:, 0])
            nc.scalar.dma_start(out=wt5[:, 1], in_=w5[:, 1])

            for n in range(IMGS):
                vt = xpool.tile([CIN, NP, VROW], f16)
                vt3 = vt.rearrange("p v (r t) -> p v r t", t=T)
                xr = x4[n].rearrange("p v (r t) -> p v r t", t=T)
                for r0, r1 in DMA_ROWS:
                    nc.sync.dma_start(out=vt3[:, :, r0:r1, :], in_=xr[:, :, r0:r1, :])

                for h in range(2):
                    for c in range(N_CHUNKS):
                        ps = pspool.tile([128, NP, 512], f32, name="ps")
                        # p order (1,2,0,3): the ScalarE copies of m1/m2 can
                        # start while the p0/p3 matmuls still run.
                        for p in (1, 2, 0, 3):
                            for dy in range(3):
                                row0 = c * ROWS_PER_CHUNK + dy
                                nc.tensor.matmul(
                                    ps[:, p, 0:FD],
                                    lhsT=wt5[:, h, dy, p, :],
                                    rhs=vt[:, p, row0 * T : row0 * T + FD],
                                    start=(dy == 0),
                                    stop=(dy == 2),
                                )
                        c1 = cpool.tile([128, FD], f16, name="c1")
                        c2 = cpool.tile([128, FD], f16, name="c2")
                        nc.scalar.copy(out=c1[:], in_=ps[:, 1, 0:FD])
                        nc.scalar.copy(out=c2[:], in_=ps[:, 2, 0:FD])
                        s = spool.tile([128, FD], f16, name="s")
                        d = spool.tile([128, FD], f16, name="d")
                        ot = opool.tile([128, 2, FD], f16, name="ot")
                        nc.vector.tensor_add(s[:], c1[:], c2[:])
                        nc.vector.tensor_add(ot[:, 0, :], s[:], ps[:, 0, 0:FD])
                        nc.vector.tensor_sub(d[:], c1[:], c2[:])
                        nc.vector.tensor_sub(ot[:, 1, :], d[:], ps[:, 3, 0:FD])
                        cs = slice(c * FD, (c + 1) * FD)
                        hs = slice(h * 128, (h + 1) * 128)
                        nc.scalar.dma_start(out=out4[n, hs, 0, cs], in_=ot[:, 0, :])
                        nc.sync.dma_start(out=out4[n, hs, 1, cs], in_=ot[:, 1, :])

    _split_sync_waits(nc, mybir)
    return nc


def _prep_inputs(input_batch, weights):
    x = np.asarray(input_batch, dtype=np.float32)
    wf = np.asarray(weights, dtype=np.float32)
    xp = np.zeros((N_FULL, CIN, HP, HP), np.float32)
    xp[:, :, 1:-1, 1:-1] = x
    V = np.empty((N_FULL, CIN, NP, HP, T), np.float32)
    V[:, :, 0] = xp[:, :, :, 0:56:2] - xp[:, :, :, 2:58:2]
    V[:, :, 1] = xp[:, :, :, 1:57:2] + xp[:, :, :, 2:58:2]
    V[:, :, 2] = xp[:, :, :, 2:58:2] - xp[:, :, :, 1:57:2]
    V[:, :, 3] = xp[:, :, :, 1:57:2] - xp[:, :, :, 3:58:2]
    V16 = V.reshape(N_FULL, CIN, NP * VROW).astype(np.float16)

    g0, g1, g2 = wf[..., 0], wf[..., 1], wf[..., 2]  # (COUT, CIN, 3[dy])
    U = np.stack([g0, (g0 + g1 + g2) / 2, (g0 - g1 + g2) / 2, g2], axis=-1)
    # U: (COUT, CIN, dy, p) -> layout [ci, h, dy, p, c]
    wt = np.ascontiguousarray(
        U.reshape(2, 128, CIN, 3, NP)
        .transpose(2, 0, 3, 4, 1)
        .reshape(CIN, 2 * 3 * NP * 128)
        .astype(np.float16)
    )
    in_maps = []
    for i in range(N_CORES):
        in_maps.append(
            {"x": np.ascontiguousarray(V16[i * IMGS : (i + 1) * IMGS]), "w": wt}
        )
    return in_maps


def _postprocess(raw):
    # raw: (IMGS, COUT, 2*1568) f16, plane-major -> (IMGS, COUT, 56, 56) f32
    y = raw.reshape(IMGS, COUT, 2, H, T).transpose(0, 1, 3, 4, 2)
    return y.reshape(IMGS, COUT, H, W).astype(np.float32)


def _run(input_batch, weights, trace=False):
    from concourse.bass_utils import run_bass_kernel_spmd

    if "nc" not in _CACHE:
        _CACHE["nc"] = _build()
    nc = _CACHE["nc"]
    in_maps = _prep_inputs(np.asarray(input_batch), np.asarray(weights))
    res = run_bass_kernel_spmd(nc, in_maps, list(range(N_CORES)), trace=trace)
    outs = [_postprocess(res.results[i]["out"]) for i in range(N_CORES)]
    full = np.concatenate(outs, axis=0)
    return full, res


def kernel(input_batch, weights):
    full, _ = _run(input_batch, weights, trace=False)
    return full


# revision 3
# speedup vs baseline: 1.3432x; 1.3082x over previous
"""Conv2D 3x3 (NCHW, OIHW, stride 1, pad 1) on 8 Trainium2 NeuronCores.

Problem shape: input (32, 128, 56, 56) fp32, weights (256, 128, 3, 3) fp32,
output (32, 256, 56, 56) fp32.

Strategy (v2 — width-axis Winograd F(2,3)):
  - Data-parallel over batch: 4 images per core, weights replicated.
  - Host applies the 1D Winograd F(2,3) input transform along W to the
    zero-padded image: for each padded row r (58 rows) and tile t (28
    2-wide output tiles), V0=d[2t]-d[2t+2], V1=d[2t+1]+d[2t+2],
    V2=d[2t+2]-d[2t+1], V3=d[2t+1]-d[2t+3], stored as 4 fp16 planes of
    [ci, 58*28].  Weights become U[dy,p] = G @ w-taps (G the F(2,3)
    weight transform), fp16.
  - Device: per image, co-half h, and 14-row chunk c, accumulate
        m_p[co, 392] = sum_dy U[h,dy,p][ci,co].T @ V_p[ci, rows 14c+dy]
    (12 matmuls, free dim 392, contract 128) into one PSUM bank per p.
    This is 1.5x fewer PE columns than direct 9-tap conv.
  - Output transform fused into the PSUM drain: ScalarE copies m1,m2 to
    SBUF fp16; VectorE computes Y0 = (m1+m2)+m0 and Y1 = (m1-m2)-m3,
    writing fp16 output planes that DMA out.  Host interleaves the two
    w-phase planes and upcasts to fp32.
"""

import sys

sys.path.insert(0, "/opt/trn_rl_repo")

import numpy as np

N_CORES = 8
N_FULL = 32
IMGS = N_FULL // N_CORES  # images per core
CIN = 128
COUT = 256
H = W = 56
HP = 58  # padded rows
T = 28  # winograd tiles per row (2 output cols each)
NP = 4  # winograd positions per tile
VROW = HP * T  # 1624 elements per V plane
ROWS_PER_CHUNK = 14
N_CHUNKS = H // ROWS_PER_CHUNK  # 4
FD = ROWS_PER_CHUNK * T  # 392 moving elements per matmul
PIX = H * W  # 3136

_CACHE = {}


def _split_sync_waits(nc, mybir, max_waits=1):
    """The walrus build in this container rejects instructions carrying
    more than one semaphore wait; hoist extras onto preceding NOPs on the
    same engine (engine executes them in order, semantics preserved)."""
    ctr = 0
    for f in nc.m.functions:
        for bb in f.blocks:
            new_insts = []
            for ins in bb.instructions:
                si = getattr(ins, "sync_info", None)
                if si is not None and si.on_wait and len(si.on_wait) > max_waits:
                    waits = list(si.on_wait)
                    extra, keep = waits[:-max_waits], waits[-max_waits:]
                    for i in range(0, len(extra), max_waits):
                        ctr += 1
                        nop = mybir.InstNoOp(
                            name=f"{ins.name}_wsplit{ctr}",
                            engine=ins.engine,
                            sync_info=mybir.SyncInfo(
                                on_wait=extra[i : i + max_waits], on_update=[]
                            ),
                            bass_nofuse=True,
                        )
                        new_insts.append(nop)
                    si.on_wait = keep
                new_insts.append(ins)
            bb.instructions[:] = new_insts
    return ctr


# input V-plane row ranges DMA'd per piece (lead piece first so chunk 0
# can start as early as possible)
DMA_ROWS = ((0, 16), (16, 30), (30, 44), (44, 58))


def _build():
    import concourse.bass as bass
    import concourse.mybir as mybir
    import concourse.tile as tile

    f32 = mybir.dt.float32
    f16 = mybir.dt.float16

    nc = bass.Bass()
    x = nc.declare_dram_parameter("x", [IMGS, CIN, NP * VROW], f16, isOutput=False)
    w = nc.declare_dram_parameter("w", [CIN, 2 * 3 * NP * 128], f16, isOutput=False)
    out = nc.declare_dram_parameter("out", [IMGS, COUT, 2 * PIX // 2], f16, isOutput=True)

    x4 = x.rearrange("n p (v q) -> n p v q", v=NP)  # q = 1624 (row*28)
    w5 = w.rearrange("p (h y v c) -> p h y v c", h=2, y=3, v=NP)
    out4 = out.rearrange("n c (v q) -> n c v q", v=2)  # q = 1568 (row*28)

    with tile.TileContext(nc) as tc:
        with (
            tc.tile_pool(name="wpool", bufs=1) as wpool,
            tc.tile_pool(name="xpool", bufs=2) as xpool,
            tc.tile_pool(name="cpool", bufs=4) as cpool,
            tc.tile_pool(name="spool", bufs=4) as spool,
            tc.tile_pool(name="opool", bufs=4) as opool,
            tc.tile_pool(name="psum", bufs=1, space="PSUM") as pspool,
        ):
            # One 8-bank PSUM tile, manually rotated: chunk parity q uses
            # banks 4q..4q+3 (one per winograd position p).  Slicing a single
            # tile gives per-bank dependency tracking, so the next chunk's
            # matmuls only wait for the reader of the specific bank they
            # write, not for the whole 4-bank group (tile-pool rotation
            # stalled the PE ~640ns per chunk).
            psa = pspool.tile([128, 8, 512], f32, name="psa")

            # PE warmup: dummy matmuls while the first DMAs are in flight so
            # HAM un-throttles (1.2->2.4 GHz) before the real matmuls start.
            # 20 x N=256 cold matmuls (~213ns each) bridge the gap until the
            # first real chunk's operands have landed -- an idle gap between
            # warmup and the real stream lets the free-running HAM activity
            # window re-arm and keeps the PE at 1.2 GHz for ~10us (measured).
            warm = wpool.tile([128, 256], f16, name="warm")
            nc.vector.memzero(warm[:])
            for _ in range(20):
                nc.tensor.matmul(
                    psa[:, 7, 0:256], lhsT=warm[:, 0:128], rhs=warm[:],
                    start=True, stop=True,
                )

            # weights on the scalar HWDGE ring (h0 first so chunk-0 h0
            # matmuls start earliest); images on the sync ring.
            wt = wpool.tile([CIN, 2 * 3 * NP * 128], f16)
            wt5 = wt.rearrange("p (h y v c) -> p h y v c", h=2, y=3, v=NP)
            nc.scalar.dma_start(out=wt5[:, 0], in_=w5[:, 0])
            nc.scalar.dma_start(out=wt5[:, 1], in_=w5[:, 1])

            def load_image(n):
                vt = xpool.tile([CIN, NP, VROW], f16)
                vt3 = vt.rearrange("p v (r t) -> p v r t", t=T)
                xr = x4[n].rearrange("p v (r t) -> p v r t", t=T)
                for r0, r1 in DMA_ROWS:
                    nc.sync.dma_start(out=vt3[:, :, r0:r1, :], in_=xr[:, :, r0:r1, :])
                return vt

            vts = {0: load_image(0)}
            chunk_idx = 0
            for n in range(IMGS):
                # prefetch next image first so its DMAs issue (and stream)
                # while this image computes
                if n + 1 < IMGS:
                    vts[n + 1] = load_image(n + 1)
                vt = vts.pop(n)
                for h in range(2):
                    for c in range(N_CHUNKS):
                        q = 4 * (chunk_idx % 2)
                        chunk_idx += 1
                        ps = psa[:, q : q + NP, :]
                        # p order (1,2,0,3): the ScalarE copies of m1/m2 can
                        # start while the p0/p3 matmuls still run.
                        for p in (1, 2, 0, 3):
                            for dy in range(3):
                                row0 = c * ROWS_PER_CHUNK + dy
                                nc.tensor.matmul(
                                    ps[:, p, 0:FD],
                                    lhsT=wt5[:, h, dy, p, :],
                                    rhs=vt[:, p, row0 * T : row0 * T + FD],
                                    start=(dy == 0),
                                    stop=(dy == 2),
                                )
                        c1 = cpool.tile([128, FD], f16, name="c1")
                        c2 = cpool.tile([128, FD], f16, name="c2")
                        nc.scalar.copy(out=c1[:], in_=ps[:, 1, 0:FD])
                        nc.scalar.copy(out=c2[:], in_=ps[:, 2, 0:FD])
                        s = spool.tile([128, FD], f16, name="s")
                        d = spool.tile([128, FD], f16, name="d")
                        ot = opool.tile([128, 2, FD], f16, name="ot")
                        # s = m1+m2 on the (otherwise idle) GPSIMD engine --
                        # both inputs are SBUF fp16 which GPSIMD can reach;
                        # keeps the DVE for the PSUM-reading ops.
                        nc.gpsimd.tensor_add(s[:], c1[:], c2[:])
                        nc.vector.tensor_add(ot[:, 0, :], s[:], ps[:, 0, 0:FD])
                        nc.vector.tensor_sub(d[:], c1[:], c2[:])
                        nc.vector.tensor_sub(ot[:, 1, :], d[:], ps[:, 3, 0:FD])
                        cs = slice(c * FD, (c + 1) * FD)
                        hs = slice(h * 128, (h + 1) * 128)
                        nc.sync.dma_start(out=out4[n, hs, :, cs], in_=ot[:, :, :])

    _split_sync_waits(nc, mybir)
    return nc


def _prep_inputs(input_batch, weights):
    x = np.asarray(input_batch, dtype=np.float32)
    wf = np.asarray(weights, dtype=np.float32)
    xp = np.zeros((N_FULL, CIN, HP, HP), np.float32)
    xp[:, :, 1:-1, 1:-1] = x
    V = np.empty((N_FULL, CIN, NP, HP, T), np.float32)
    V[:, :, 0] = xp[:, :, :, 0:56:2] - xp[:, :, :, 2:58:2]
    V[:, :, 1] = xp[:, :, :, 1:57:2] + xp[:, :, :, 2:58:2]
    V[:, :, 2] = xp[:, :, :, 2:58:2] - xp[:, :, :, 1:57:2]
    V[:, :, 3] = xp[:, :, :, 1:57:2] - xp[:, :, :, 3:58:2]
    V16 = V.reshape(N_FULL, CIN, NP * VROW).astype(np.float16)

    g0, g1, g2 = wf[..., 0], wf[..., 1], wf[..., 2]  # (COUT, CIN, 3[dy])
    U = np.stack([g0, (g0 + g1 + g2) / 2, (g0 - g1 + g2) / 2, g2], axis=-1)
    # U: (COUT, CIN, dy, p) -> layout [ci, h, dy, p, c]
    wt = np.ascontiguousarray(
        U.reshape(2, 128, CIN, 3, NP)
        .transpose(2, 0, 3, 4, 1)
        .reshape(CIN, 2 * 3 * NP * 128)
        .astype(np.float16)
    )
    in_maps = []
    for i in range(N_CORES):
        in_maps.append(
            {"x": np.ascontiguousarray(V16[i * IMGS : (i + 1) * IMGS]), "w": wt}
        )
    return in_maps


def _postprocess(raw):
    # raw: (IMGS, COUT, 2*1568) f16, plane-major -> (IMGS, COUT, 56, 56) f32
    y = raw.reshape(IMGS, COUT, 2, H, T).transpose(0, 1, 3, 4, 2)
    return y.reshape(IMGS, COUT, H, W).astype(np.float32)


def _run(input_batch, weights, trace=False):
    from concourse.bass_utils import run_bass_kernel_spmd

    if "nc" not in _CACHE:
        _CACHE["nc"] = _build()
    nc = _CACHE["nc"]
    in_maps = _prep_inputs(np.asarray(input_batch), np.asarray(weights))
    res = run_bass_kernel_spmd(nc, in_maps, list(range(N_CORES)), trace=trace)
    outs = [_postprocess(res.results[i]["out"]) for i in range(N_CORES)]
    full = np.concatenate(outs, axis=0)
    return full, res


def kernel(input_batch, weights):
    full, _ = _run(input_batch, weights, trace=False)
    return full


# revision 7
# speedup vs baseline: 1.3846x; 1.0308x over previous
"""Conv2D 3x3 (NCHW, OIHW, stride 1, pad 1) on 8 Trainium2 NeuronCores.

Problem shape: input (32, 128, 56, 56) fp32, weights (256, 128, 3, 3) fp32,
output (32, 256, 56, 56) fp32.

Strategy (v2 — width-axis Winograd F(2,3)):
  - Data-parallel over batch: 4 images per core, weights replicated.
  - Host applies the 1D Winograd F(2,3) input transform along W to the
    zero-padded image: for each padded row r (58 rows) and tile t (28
    2-wide output tiles), V0=d[2t]-d[2t+2], V1=d[2t+1]+d[2t+2],
    V2=d[2t+2]-d[2t+1], V3=d[2t+1]-d[2t+3], stored as 4 fp16 planes of
    [ci, 58*28].  Weights become U[dy,p] = G @ w-taps (G the F(2,3)
    weight transform), fp16.
  - Device: per image, co-half h, and 14-row chunk c, accumulate
        m_p[co, 392] = sum_dy U[h,dy,p][ci,co].T @ V_p[ci, rows 14c+dy]
    (12 matmuls, free dim 392, contract 128) into one PSUM bank per p.
    This is 1.5x fewer PE columns than direct 9-tap conv.
  - Output transform fused into the PSUM drain: ScalarE copies m1,m2 to
    SBUF fp16; VectorE computes Y0 = (m1+m2)+m0 and Y1 = (m1-m2)-m3,
    writing fp16 output planes that DMA out.  Host interleaves the two
    w-phase planes and upcasts to fp32.
"""

import sys

sys.path.insert(0, "/opt/trn_rl_repo")

import numpy as np

N_CORES = 8
N_FULL = 32
IMGS = N_FULL // N_CORES  # images per core
CIN = 128
COUT = 256
H = W = 56
HP = 58  # padded rows
T = 28  # winograd tiles per row (2 output cols each)
NP = 4  # winograd positions per tile
VROW = HP * T  # 1624 elements per V plane
ROWS_PER_CHUNK = 14
N_CHUNKS = H // ROWS_PER_CHUNK  # 4
FD = ROWS_PER_CHUNK * T  # 392 moving elements per matmul
PIX = H * W  # 3136

_CACHE = {}


def _split_sync_waits(nc, mybir, max_waits=1):
    """The walrus build in this container rejects instructions carrying
    more than one semaphore wait; hoist extras onto preceding NOPs on the
    same engine (engine executes them in order, semantics preserved)."""
    ctr = 0
    for f in nc.m.functions:
        for bb in f.blocks:
            new_insts = []
            for ins in bb.instructions:
                si = getattr(ins, "sync_info", None)
                if si is not None and si.on_wait and len(si.on_wait) > max_waits:
                    waits = list(si.on_wait)
                    extra, keep = waits[:-max_waits], waits[-max_waits:]
                    for i in range(0, len(extra), max_waits):
                        ctr += 1
                        nop = mybir.InstNoOp(
                            name=f"{ins.name}_wsplit{ctr}",
                            engine=ins.engine,
                            sync_info=mybir.SyncInfo(
                                on_wait=extra[i : i + max_waits], on_update=[]
                            ),
                            bass_nofuse=True,
                        )
                        new_insts.append(nop)
                    si.on_wait = keep
                new_insts.append(ins)
            bb.instructions[:] = new_insts
    return ctr


# input V-plane row ranges DMA'd per piece (lead piece first so chunk 0
# can start as early as possible); later images are fully prefetched an
# image ahead, so fewer/larger pieces suffice
DMA_ROWS_FIRST = ((0, 16), (16, 30), (30, 44), (44, 58))
DMA_ROWS_PREFETCH = ((0, 30), (30, 58))


def _build():
    import concourse.bass as bass
    import concourse.mybir as mybir
    import concourse.tile as tile

    f32 = mybir.dt.float32
    f16 = mybir.dt.float16
    bf16 = mybir.dt.bfloat16

    nc = bass.Bass()
    x = nc.declare_dram_parameter("x", [IMGS, CIN, NP * VROW], f16, isOutput=False)
    w = nc.declare_dram_parameter("w", [CIN, 2 * 3 * NP * 128], f16, isOutput=False)
    out = nc.declare_dram_parameter("out", [IMGS, COUT, 2 * PIX // 2], f16, isOutput=True)

    x4 = x.rearrange("n p (v q) -> n p v q", v=NP)  # q = 1624 (row*28)
    w5 = w.rearrange("p (h y v c) -> p h y v c", h=2, y=3, v=NP)
    out4 = out.rearrange("n c (v q) -> n c v q", v=2)  # q = 1568 (row*28)

    with tile.TileContext(nc) as tc:
        with (
            tc.tile_pool(name="wpool", bufs=1) as wpool,
            tc.tile_pool(name="xpool", bufs=2) as xpool,
            tc.tile_pool(name="cpool", bufs=4) as cpool,
            tc.tile_pool(name="spool", bufs=4) as spool,
            tc.tile_pool(name="opool", bufs=4) as opool,
            tc.tile_pool(name="psum", bufs=1, space="PSUM") as pspool,
        ):
            # One 8-bank PSUM tile, manually rotated: chunk parity q uses
            # banks 4q..4q+3 (one per winograd position p).  Slicing a single
            # tile gives per-bank dependency tracking, so the next chunk's
            # matmuls only wait for the reader of the specific bank they
            # write, not for the whole 4-bank group (tile-pool rotation
            # stalled the PE ~640ns per chunk).
            psa = pspool.tile([128, 8, 512], f32, name="psa")

            # PE warmup: dummy matmuls while the first DMAs are in flight so
            # HAM un-throttles (1.2->2.4 GHz) before the real matmuls start.
            # 14 x N=256 cold matmuls (~213ns each) bridge the gap until the
            # first real chunk's operands have landed -- an idle gap between
            # warmup and the real stream lets the free-running HAM activity
            # window re-arm and keeps the PE at 1.2 GHz for ~10us (measured).
            warm = wpool.tile([128, 256], f16, name="warm")
            nc.vector.memzero(warm[:])
            for _ in range(14):
                nc.tensor.matmul(
                    psa[:, 7, 0:256], lhsT=warm[:, 0:128], rhs=warm[:],
                    start=True, stop=True,
                )

            # DMA ring split: weights + outputs ride the sync HWDGE queue,
            # input planes ride the scalar queue -- one queue for all 13MB
            # saturates and the final output DMAs drain ~3us late.
            wt = wpool.tile([CIN, 2 * 3 * NP * 128], f16)
            wt5 = wt.rearrange("p (h y v c) -> p h y v c", h=2, y=3, v=NP)
            nc.sync.dma_start(out=wt5[:, 0], in_=w5[:, 0])
            nc.sync.dma_start(out=wt5[:, 1], in_=w5[:, 1])

            def load_image(n):
                vt = xpool.tile([CIN, NP, VROW], f16)
                vt3 = vt.rearrange("p v (r t) -> p v r t", t=T)
                xr = x4[n].rearrange("p v (r t) -> p v r t", t=T)
                rows = DMA_ROWS_FIRST if n == 0 else DMA_ROWS_PREFETCH
                for r0, r1 in rows:
                    nc.scalar.dma_start(out=vt3[:, :, r0:r1, :], in_=xr[:, :, r0:r1, :])
                return vt

            vts = {0: load_image(0)}
            chunk_idx = 0
            for n in range(IMGS):
                # prefetch next image first so its DMAs issue (and stream)
                # while this image computes
                if n + 1 < IMGS:
                    vts[n + 1] = load_image(n + 1)
                vt = vts.pop(n)
                for h in range(2):
                    for c in range(N_CHUNKS):
                        q = 4 * (chunk_idx % 2)
                        chunk_idx += 1
                        ps = psa[:, q : q + NP, :]
                        # p order (1,2,0,3): the ScalarE copies of m1/m2 can
                        # start while the p0/p3 matmuls still run.
                        for p in (1, 2, 0, 3):
                            for dy in range(3):
                                row0 = c * ROWS_PER_CHUNK + dy
                                nc.tensor.matmul(
                                    ps[:, p, 0:FD],
                                    lhsT=wt5[:, h, dy, p, :],
                                    rhs=vt[:, p, row0 * T : row0 * T + FD],
                                    start=(dy == 0),
                                    stop=(dy == 2),
                                )
                        c1 = cpool.tile([128, FD], bf16, name="c1")
                        c2 = cpool.tile([128, FD], bf16, name="c2")
                        nc.scalar.copy(out=c1[:], in_=ps[:, 1, 0:FD])
                        nc.scalar.copy(out=c2[:], in_=ps[:, 2, 0:FD])
                        s = spool.tile([128, FD], bf16, name="s")
                        d = spool.tile([128, FD], bf16, name="d")
                        ot = opool.tile([128, 2, FD], f16, name="ot")
                        # s = m1+m2 on the (otherwise idle) GPSIMD engine --
                        # both inputs are SBUF fp16 which GPSIMD can reach;
                        # keeps the DVE for the PSUM-reading ops.
                        nc.gpsimd.tensor_add(s[:], c1[:], c2[:])
                        nc.vector.tensor_add(ot[:, 0, :], s[:], ps[:, 0, 0:FD])
                        nc.vector.tensor_sub(d[:], c1[:], c2[:])
                        nc.vector.tensor_sub(ot[:, 1, :], d[:], ps[:, 3, 0:FD])
                        cs = slice(c * FD, (c + 1) * FD)
                        hs = slice(h * 128, (h + 1) * 128)
                        nc.sync.dma_start(out=out4[n, hs, :, cs], in_=ot[:, :, :])

    _split_sync_waits(nc, mybir)
    return nc


def _prep_inputs(input_batch, weights):
    x = np.asarray(input_batch, dtype=np.float32)
    wf = np.asarray(weights, dtype=np.float32)
    xp = np.zeros((N_FULL, CIN, HP, HP), np.float32)
    xp[:, :, 1:-1, 1:-1] = x
    V = np.empty((N_FULL, CIN, NP, HP, T), np.float32)
    V[:, :, 0] = xp[:, :, :, 0:56:2] - xp[:, :, :, 2:58:2]
    V[:, :, 1] = xp[:, :, :, 1:57:2] + xp[:, :, :, 2:58:2]
    V[:, :, 2] = xp[:, :, :, 2:58:2] - xp[:, :, :, 1:57:2]
    V[:, :, 3] = xp[:, :, :, 1:57:2] - xp[:, :, :, 3:58:2]
    V16 = V.reshape(N_FULL, CIN, NP * VROW).astype(np.float16)

    g0, g1, g2 = wf[..., 0], wf[..., 1], wf[..., 2]  # (COUT, CIN, 3[dy])
    U = np.stack([g0, (g0 + g1 + g2) / 2, (g0 - g1 + g2) / 2, g2], axis=-1)
    # U: (COUT, CIN, dy, p) -> layout [ci, h, dy, p, c]
    wt = np.ascontiguousarray(
        U.reshape(2, 128, CIN, 3, NP)
        .transpose(2, 0, 3, 4, 1)
        .reshape(CIN, 2 * 3 * NP * 128)
        .astype(np.float16)
    )
    in_maps = []
    for i in range(N_CORES):
        in_maps.append(
            {"x": np.ascontiguousarray(V16[i * IMGS : (i + 1) * IMGS]), "w": wt}
        )
    return in_maps


def _postprocess(raw):
    # raw: (IMGS, COUT, 2*1568) f16, plane-major -> (IMGS, COUT, 56, 56) f32
    y = raw.reshape(IMGS, COUT, 2, H, T).transpose(0, 1, 3, 4, 2)
    return y.reshape(IMGS, COUT, H, W).astype(np.float32)


def _run(input_batch, weights, trace=False):
    from concourse.bass_utils import run_bass_kernel_spmd

    if "nc" not in _CACHE:
        _CACHE["nc"] = _build()
    nc = _CACHE["nc"]
    in_maps = _prep_inputs(np.asarray(input_batch), np.asarray(weights))
    res = run_bass_kernel_spmd(nc, in_maps, list(range(N_CORES)), trace=trace)
    outs = [_postprocess(res.results[i]["out"]) for i in range(N_CORES)]
    full = np.concatenate(outs, axis=0)
    return full, res


def kernel(input_batch, weights):
    full, _ = _run(input_batch, weights, trace=False)
    return full
